# revision 3
# baseline (speedup 1.0000x reference)
"""AttentionPooling (segment softmax + weighted segment sum) on 8 trn2 cores.

Strategy: shard whole segments across cores (sorted batch -> contiguous node
ranges).  Host pre-casts x to bf16 and uploads ONE interleaved tensor per
core, laid out [128 part, n_tiles, 513] where per 128-node tile:
  cols [0:256]   node-partitioned x   (row p = node t*128+p)
  col  [256]     ones (yields softmax denominators in the same matmul)
  cols [257:513] channel-partitioned x (row p = channel, col j = node t*128+j)
so the device reads 64MB of contiguous bf16 per core (one ~4MB DMA per
32-tile chunk) and does no cast or transpose on chip.  Per chunk: PE computes
h = tanh(xT @ W1 + b1) (hidden-partitioned), per-tile score columns
s = h_tile.T @ W2, ACT exponentiates, DVE builds we = onehot(batch)*e, and
PE accumulates [64,257] = we.T @ [x | 1] in PSUM across all chunks
(column 256 = softmax denominators).  The tail chunk is partial (variable
tile count) so node padding is <1%.  Softmax max-subtraction is skipped:
|s| <= ||W2||_1 + |b2| ~ 28, exp stays in fp32 range.
"""

from contextlib import ExitStack

import numpy as np
import ml_dtypes

import concourse.bass as bass
import concourse.bacc as bacc
import concourse.tile as tile
from concourse import mybir
from concourse.bass_utils import run_bass_kernel_spmd

N_CORES = 8
NUM_GRAPHS = 512
SEGS_PER_CORE = NUM_GRAPHS // N_CORES  # 64
D = 256          # in channels
H = 128          # hidden
P = 128          # partitions
TILE_N = 128     # nodes per weight tile
CHUNK_T = 32     # max tiles per chunk
DW = D + 1       # node-partitioned row width: 256 channels + ones column
TW = DW + D      # total row width per tile: 257 + 256 = 513

_BF16 = mybir.dt.bfloat16
_F32 = mybir.dt.float32
_I32 = mybir.dt.int32


def _build_program(n_tiles: int, b2_val: float):
    nc = bacc.Bacc()
    chunks = [CHUNK_T] * (n_tiles // CHUNK_T)
    if n_tiles % CHUNK_T:
        chunks.append(n_tiles % CHUNK_T)
    n_chunks = len(chunks)

    xin_d = nc.declare_dram_parameter("xin", [P, n_tiles * TW], _BF16,
                                      isOutput=False)
    bt_d = nc.declare_dram_parameter("batch_t", [P, n_tiles + SEGS_PER_CORE],
                                     _I32, isOutput=False)
    w1_d = nc.declare_dram_parameter("w1", [D, H], _BF16, isOutput=False)
    w2_d = nc.declare_dram_parameter("w2", [H, 1], _BF16, isOutput=False)
    b1_d = nc.declare_dram_parameter("b1", [H, 1], _F32, isOutput=False)
    out_d = nc.declare_dram_parameter("out_g", [SEGS_PER_CORE, D], _F32,
                                      isOutput=True)

    xin_ap = xin_d[:].rearrange("p (t w) -> p t w", w=TW)

    with tile.TileContext(nc) as tc, ExitStack() as ctx:
        const_pool = ctx.enter_context(tc.tile_pool(name="consts", bufs=1))
        x_pool = ctx.enter_context(tc.tile_pool(name="xin", bufs=3))
        h_pool = ctx.enter_context(tc.tile_pool(name="h", bufs=2))
        cmp_pool = ctx.enter_context(tc.tile_pool(name="cmp", bufs=2))
        we_pool = ctx.enter_context(tc.tile_pool(name="we", bufs=2))
        ecol_pool = ctx.enter_context(tc.tile_pool(name="ecol", bufs=2))
        fin_pool = ctx.enter_context(tc.tile_pool(name="fin", bufs=1))
        psum_h = ctx.enter_context(
            tc.tile_pool(name="psum_h", bufs=2, space=bass.MemorySpace.PSUM))
        psum_s = ctx.enter_context(
            tc.tile_pool(name="psum_s", bufs=2, space=bass.MemorySpace.PSUM))
        psum_acc = ctx.enter_context(
            tc.tile_pool(name="psum_acc", bufs=1, space=bass.MemorySpace.PSUM))

        # ---- constants / weights ----
        w1_sb = const_pool.tile([P, 2, H], _BF16, tag="w1")   # [:, 0, :]=ch 0-127
        nc.sync.dma_start(w1_sb[:, 0, :], w1_d[0:128, :])
        nc.sync.dma_start(w1_sb[:, 1, :], w1_d[128:256, :])
        w2_sb = const_pool.tile([P, 1], _BF16, tag="w2")
        nc.sync.dma_start(w2_sb[:], w2_d[:])
        b1_sb = const_pool.tile([P, 1], _F32, tag="b1")
        nc.sync.dma_start(b1_sb[:], b1_d[:])
        bt_sb = const_pool.tile([P, n_tiles + SEGS_PER_CORE], _I32, tag="bt")
        nc.sync.dma_start(bt_sb[:], bt_d[:])
        iota_sb = bt_sb[:, n_tiles:n_tiles + SEGS_PER_CORE]

        acc_ps = psum_acc.tile([SEGS_PER_CORE, DW], _F32, tag="acc")

        saved = {}

        def emit_load_h(c, t0, tc_):
            x_t = x_pool.tile([P, CHUNK_T, TW], _BF16, tag="xin")
            dma = nc.sync if c % 2 == 0 else nc.scalar
            dma.dma_start(x_t[:, 0:tc_, :], xin_ap[:, t0:t0 + tc_, :])

            # h = tanh(x @ W1 + b1), hidden-partitioned, bf16
            h_bf = h_pool.tile([P, CHUNK_T * TILE_N], _BF16, tag="h")
            for s0 in range(0, tc_, 4):
                sn = min(4, tc_ - s0)
                ph = psum_h.tile([P, 512], _F32, tag="ph")
                nc.tensor.matmul(ph[:, 0:sn * TILE_N], w1_sb[:, 0, :],
                                 x_t[:, s0:s0 + sn, DW:DW + H],
                                 start=True, stop=False)
                nc.tensor.matmul(ph[:, 0:sn * TILE_N], w1_sb[:, 1, :],
                                 x_t[:, s0:s0 + sn, DW + H:TW],
                                 start=False, stop=True)
                nc.scalar.activation(
                    h_bf[:, s0 * TILE_N:(s0 + sn) * TILE_N],
                    ph[:, 0:sn * TILE_N],
                    mybir.ActivationFunctionType.Tanh, bias=b1_sb[:])
            saved[c] = (x_t, h_bf)

        def emit_score(c, t0, tc_):
            x_t, h_bf = saved[c]
            # per-tile score columns: s_col[p, t] = h_tile.T @ W2
            ps_s = psum_s.tile([P, CHUNK_T], _F32, tag="ps_s")
            for t in range(tc_):
                nc.tensor.matmul(ps_s[:, t:t + 1],
                                 h_bf[:, t * TILE_N:(t + 1) * TILE_N],
                                 w2_sb, start=True, stop=True)

            # e = exp(s + b2)  (node-partitioned, fp32)
            e_col = ecol_pool.tile([P, CHUNK_T], _F32, tag="ecol")
            nc.scalar.activation(e_col[:, 0:tc_], ps_s[:, 0:tc_],
                                 mybir.ActivationFunctionType.Exp,
                                 bias=float(b2_val))

            # we[p, t, g] = (batch_t == g) * e   (bf16)
            cmp = cmp_pool.tile([P, CHUNK_T, SEGS_PER_CORE], _BF16, tag="cmp")
            bt_c = bt_sb[:, t0:t0 + tc_]
            nc.vector.tensor_tensor(
                cmp[:, 0:tc_],
                bt_c.unsqueeze(2).broadcast_to([P, tc_, SEGS_PER_CORE]),
                iota_sb.unsqueeze(1).broadcast_to([P, tc_, SEGS_PER_CORE]),
                mybir.AluOpType.is_equal)
            we = we_pool.tile([P, CHUNK_T, SEGS_PER_CORE], _BF16, tag="we")
            nc.vector.tensor_tensor(
                we[:, 0:tc_], cmp[:, 0:tc_],
                e_col[:, 0:tc_].unsqueeze(2).broadcast_to(
                    [P, tc_, SEGS_PER_CORE]),
                mybir.AluOpType.mult)
            saved[c] = (x_t, we)

        def emit_wsum(c, tc_, first, last):
            x_t, we = saved.pop(c)
            for t in range(tc_):
                nc.tensor.matmul(acc_ps[:], we[:, t, :], x_t[:, t, 0:DW],
                                 start=(first and t == 0),
                                 stop=(last and t == tc_ - 1),
                                 skip_group_check=True)

        t0s = np.concatenate([[0], np.cumsum(chunks)]).astype(int)
        for c in range(n_chunks):
            emit_load_h(c, int(t0s[c]), chunks[c])
            if c >= 1:
                emit_wsum(c - 1, chunks[c - 1], c - 1 == 0, False)
            emit_score(c, int(t0s[c]), chunks[c])
        emit_wsum(n_chunks - 1, chunks[-1], n_chunks == 1, True)

        # ---- epilogue: out = acc[:, :256] / acc[:, 256] ----
        den_sb = fin_pool.tile([SEGS_PER_CORE, 1], _F32, tag="den_sb")
        nc.vector.tensor_scalar_add(den_sb[:], acc_ps[:, D:DW], 1e-30)
        rec_sb = fin_pool.tile([SEGS_PER_CORE, 1], _F32, tag="rec_sb")
        nc.vector.reciprocal(rec_sb[:], den_sb[:])
        out_sb = fin_pool.tile([SEGS_PER_CORE, D], _F32, tag="out_sb")
        nc.vector.tensor_scalar_mul(out_sb[:], acc_ps[:, 0:D], rec_sb[:])
        nc.sync.dma_start(out_d[:], out_sb[:])

    return nc


def _prepare_inputs(x, W1, b1, W2, b2, batch):
    x = np.asarray(x)
    batch = np.asarray(batch).astype(np.int64)
    # core k owns segments [64k, 64(k+1)); sorted batch -> contiguous ranges
    bounds = np.searchsorted(batch, np.arange(0, NUM_GRAPHS + 1, SEGS_PER_CORE))
    counts = np.diff(bounds)
    nmax = int(np.max(counts))
    n_tiles = max(1, (nmax + TILE_N - 1) // TILE_N)
    nmax_pad = n_tiles * TILE_N

    x_bf = x.astype(ml_dtypes.bfloat16)
    w1_bf = np.asarray(W1, np.float32).astype(ml_dtypes.bfloat16)
    w2_bf = np.asarray(W2, np.float32).reshape(H, 1).astype(ml_dtypes.bfloat16)
    b1_col = np.asarray(b1, np.float32).reshape(H, 1)

    in_maps = []
    for k in range(N_CORES):
        lo, hi = int(bounds[k]), int(bounds[k + 1])
        cnt = hi - lo
        xp = np.zeros((nmax_pad, D), ml_dtypes.bfloat16)
        xp[:cnt] = x_bf[lo:hi]
        xin = np.empty((P, n_tiles, TW), ml_dtypes.bfloat16)
        # node-partitioned [p, t, ch] + ones column
        xin[:, :, 0:D] = xp.reshape(n_tiles, P, D).transpose(1, 0, 2)
        xin[:, :, D] = 1.0
        # channel-partitioned [p(ch in half), t, half, node]
        xin[:, :, DW:TW] = xp.reshape(n_tiles, TILE_N, 2, P).transpose(
            3, 0, 2, 1).reshape(P, n_tiles, D)

        bt = np.full((nmax_pad,), -1, np.int32)
        bt[:cnt] = batch[lo:hi] - k * SEGS_PER_CORE
        bt_t = bt.reshape(n_tiles, P).T  # (128, n_tiles)
        iota_cols = np.tile(np.arange(SEGS_PER_CORE, dtype=np.int32), (P, 1))
        bt_t = np.concatenate([bt_t, iota_cols], axis=1).copy()
        in_maps.append({
            "xin": xin.reshape(P, n_tiles * TW),
            "batch_t": bt_t,
            "w1": w1_bf,
            "w2": w2_bf,
            "b1": b1_col,
        })
    return in_maps, n_tiles


def run(x, W1, b1, W2, b2, batch, trace=False, trace_kwargs=None):
    in_maps, n_tiles = _prepare_inputs(x, W1, b1, W2, b2, batch)
    nc = _build_program(n_tiles, float(np.asarray(b2).reshape(-1)[0]))
    nc.finalize()
    res = run_bass_kernel_spmd(nc, in_maps, list(range(N_CORES)),
                               trace=trace, **(trace_kwargs or {}))
    out = np.concatenate([np.asarray(res.results[k]["out_g"], np.float32)
                          for k in range(N_CORES)], axis=0)
    return out, res


def kernel(x, W1, b1, W2, b2, batch):
    out, _ = run(x, W1, b1, W2, b2, batch)
    return out


# revision 8
# speedup vs baseline: 1.0255x; 1.0255x over previous
"""AttentionPooling (segment softmax + weighted segment sum) on 8 trn2 cores.

Strategy: shard whole segments across cores (sorted batch -> contiguous node
ranges).  Host pre-casts x to bf16 and uploads ONE interleaved tensor per
core, laid out [128 part, n_tiles, 513] where per 128-node tile:
  cols [0:256]   node-partitioned x   (row p = node t*128+p)
  col  [256]     ones (yields softmax denominators in the same matmul)
  cols [257:513] channel-partitioned x (row p = channel, col j = node t*128+j)
so the device reads 64MB of contiguous bf16 per core (one ~4MB DMA per
32-tile chunk) and does no cast or transpose on chip.  Per chunk: PE computes
h = tanh(xT @ W1 + b1) (hidden-partitioned), per-tile score columns
s = h_tile.T @ W2, ACT exponentiates, DVE builds we = onehot(batch)*e, and
PE accumulates [64,257] = we.T @ [x | 1] in PSUM across all chunks
(column 256 = softmax denominators).  The tail chunk is partial (variable
tile count) so node padding is <1%.  Softmax max-subtraction is skipped:
|s| <= ||W2||_1 + |b2| ~ 28, exp stays in fp32 range.
"""

from contextlib import ExitStack

import numpy as np
import ml_dtypes

import concourse.bass as bass
import concourse.bacc as bacc
import concourse.tile as tile
from concourse import mybir
from concourse.bass_utils import run_bass_kernel_spmd

N_CORES = 8
NUM_GRAPHS = 512
SEGS_PER_CORE = NUM_GRAPHS // N_CORES  # 64
D = 256          # in channels
H = 128          # hidden
P = 128          # partitions
TILE_N = 128     # nodes per weight tile
CHUNK_T = 32     # max tiles per chunk
DW = D + 1       # node-partitioned row width: 256 channels + ones column
TW = DW + D      # total row width per tile: 257 + 256 = 513

_BF16 = mybir.dt.bfloat16
_F32 = mybir.dt.float32
_I32 = mybir.dt.int32


def _build_program(n_tiles: int, b2_val: float):
    nc = bacc.Bacc()
    chunks = [CHUNK_T] * (n_tiles // CHUNK_T)
    if n_tiles % CHUNK_T:
        chunks.append(n_tiles % CHUNK_T)
    n_chunks = len(chunks)

    n_full = n_tiles // CHUNK_T
    xin_d = None
    if n_full:
        xin_d = nc.declare_dram_parameter(
            "xin", [n_full * P, CHUNK_T * TW], _BF16, isOutput=False)
    xtail_d = None
    if n_tiles % CHUNK_T:
        xtail_d = nc.declare_dram_parameter(
            "xtail", [P, (n_tiles % CHUNK_T) * TW], _BF16, isOutput=False)
    bt_d = nc.declare_dram_parameter("batch_t", [P, n_tiles + SEGS_PER_CORE],
                                     _I32, isOutput=False)
    w1_d = nc.declare_dram_parameter("w1", [D, H], _BF16, isOutput=False)
    w2_d = nc.declare_dram_parameter("w2", [H, 1], _BF16, isOutput=False)
    b1_d = nc.declare_dram_parameter("b1", [H, 1], _F32, isOutput=False)
    out_d = nc.declare_dram_parameter("out_g", [SEGS_PER_CORE, D], _F32,
                                      isOutput=True)

    xin_ap = None
    if xin_d is not None:
        xin_ap = xin_d[:].rearrange("(c p) (t w) -> c p t w", p=P, w=TW)
    xtail_ap = None
    if xtail_d is not None:
        xtail_ap = xtail_d[:].rearrange("p (t w) -> p t w", w=TW)

    with tile.TileContext(nc) as tc, ExitStack() as ctx:
        const_pool = ctx.enter_context(tc.tile_pool(name="consts", bufs=1))
        x_pool = ctx.enter_context(tc.tile_pool(name="xin", bufs=3))
        h_pool = ctx.enter_context(tc.tile_pool(name="h", bufs=2))
        cmp_pool = ctx.enter_context(tc.tile_pool(name="cmp", bufs=2))
        we_pool = ctx.enter_context(tc.tile_pool(name="we", bufs=2))
        ecol_pool = ctx.enter_context(tc.tile_pool(name="ecol", bufs=2))
        fin_pool = ctx.enter_context(tc.tile_pool(name="fin", bufs=1))
        psum_h = ctx.enter_context(
            tc.tile_pool(name="psum_h", bufs=2, space=bass.MemorySpace.PSUM))
        psum_s = ctx.enter_context(
            tc.tile_pool(name="psum_s", bufs=2, space=bass.MemorySpace.PSUM))
        psum_acc = ctx.enter_context(
            tc.tile_pool(name="psum_acc", bufs=1, space=bass.MemorySpace.PSUM))

        # ---- constants / weights ----
        w1_sb = const_pool.tile([P, 2, H], _BF16, tag="w1")   # [:, 0, :]=ch 0-127
        nc.sync.dma_start(w1_sb[:, 0, :], w1_d[0:128, :])
        nc.sync.dma_start(w1_sb[:, 1, :], w1_d[128:256, :])
        w2_sb = const_pool.tile([P, 1], _BF16, tag="w2")
        nc.sync.dma_start(w2_sb[:], w2_d[:])
        b1_sb = const_pool.tile([P, 1], _F32, tag="b1")
        nc.sync.dma_start(b1_sb[:], b1_d[:])
        bt_sb = const_pool.tile([P, n_tiles + SEGS_PER_CORE], _I32, tag="bt")
        nc.sync.dma_start(bt_sb[:], bt_d[:])
        iota_sb = bt_sb[:, n_tiles:n_tiles + SEGS_PER_CORE]

        acc_ps = psum_acc.tile([SEGS_PER_CORE, DW], _F32, tag="acc")

        saved = {}

        def emit_load_h(c, t0, tc_):
            x_t = x_pool.tile([P, CHUNK_T, TW], _BF16, tag="xin")
            dma = nc.sync if c % 2 == 0 else nc.scalar
            src = xin_ap[c] if c < n_full else xtail_ap
            dma.dma_start(x_t[:, 0:tc_, :], src)

            # h = tanh(x @ W1 + b1), hidden-partitioned, bf16
            h_bf = h_pool.tile([P, CHUNK_T * TILE_N], _BF16, tag="h")
            for s0 in range(0, tc_, 4):
                sn = min(4, tc_ - s0)
                ph = psum_h.tile([P, 512], _F32, tag="ph")
                nc.tensor.matmul(ph[:, 0:sn * TILE_N], w1_sb[:, 0, :],
                                 x_t[:, s0:s0 + sn, DW:DW + H],
                                 start=True, stop=False)
                nc.tensor.matmul(ph[:, 0:sn * TILE_N], w1_sb[:, 1, :],
                                 x_t[:, s0:s0 + sn, DW + H:TW],
                                 start=False, stop=True)
                nc.scalar.activation(
                    h_bf[:, s0 * TILE_N:(s0 + sn) * TILE_N],
                    ph[:, 0:sn * TILE_N],
                    mybir.ActivationFunctionType.Tanh, bias=b1_sb[:])
            saved[c] = (x_t, h_bf)

        def emit_score(c, t0, tc_):
            x_t, h_bf = saved[c]
            # per-tile score columns: s_col[p, t] = h_tile.T @ W2
            ps_s = psum_s.tile([P, CHUNK_T], _F32, tag="ps_s")
            for t in range(tc_):
                nc.tensor.matmul(ps_s[:, t:t + 1],
                                 h_bf[:, t * TILE_N:(t + 1) * TILE_N],
                                 w2_sb, start=True, stop=True)

            # e = exp(s + b2)  (node-partitioned, fp32)
            e_col = ecol_pool.tile([P, CHUNK_T], _F32, tag="ecol")
            nc.scalar.activation(e_col[:, 0:tc_], ps_s[:, 0:tc_],
                                 mybir.ActivationFunctionType.Exp,
                                 bias=float(b2_val))

            # we[p, t, g] = (batch_t == g) * e   (bf16)
            cmp = cmp_pool.tile([P, CHUNK_T, SEGS_PER_CORE], _BF16, tag="cmp")
            bt_c = bt_sb[:, t0:t0 + tc_]
            nc.vector.tensor_tensor(
                cmp[:, 0:tc_],
                bt_c.unsqueeze(2).broadcast_to([P, tc_, SEGS_PER_CORE]),
                iota_sb.unsqueeze(1).broadcast_to([P, tc_, SEGS_PER_CORE]),
                mybir.AluOpType.is_equal)
            we = we_pool.tile([P, CHUNK_T, SEGS_PER_CORE], _BF16, tag="we")
            nc.vector.tensor_tensor(
                we[:, 0:tc_], cmp[:, 0:tc_],
                e_col[:, 0:tc_].unsqueeze(2).broadcast_to(
                    [P, tc_, SEGS_PER_CORE]),
                mybir.AluOpType.mult)
            saved[c] = (x_t, we)

        def emit_wsum(c, tc_, first, last):
            x_t, we = saved.pop(c)
            for t in range(tc_):
                nc.tensor.matmul(acc_ps[:], we[:, t, :], x_t[:, t, 0:DW],
                                 start=(first and t == 0),
                                 stop=(last and t == tc_ - 1),
                                 skip_group_check=True)

        t0s = np.concatenate([[0], np.cumsum(chunks)]).astype(int)
        for c in range(n_chunks):
            emit_load_h(c, int(t0s[c]), chunks[c])
            if c >= 1:
                emit_wsum(c - 1, chunks[c - 1], c - 1 == 0, False)
            emit_score(c, int(t0s[c]), chunks[c])
        emit_wsum(n_chunks - 1, chunks[-1], n_chunks == 1, True)

        # ---- epilogue: out = acc[:, :256] / acc[:, 256] ----
        den_sb = fin_pool.tile([SEGS_PER_CORE, 1], _F32, tag="den_sb")
        nc.vector.tensor_scalar_add(den_sb[:], acc_ps[:, D:DW], 1e-30)
        rec_sb = fin_pool.tile([SEGS_PER_CORE, 1], _F32, tag="rec_sb")
        nc.vector.reciprocal(rec_sb[:], den_sb[:])
        out_sb = fin_pool.tile([SEGS_PER_CORE, D], _F32, tag="out_sb")
        nc.vector.tensor_scalar_mul(out_sb[:], acc_ps[:, 0:D], rec_sb[:])
        nc.sync.dma_start(out_d[:], out_sb[:])

    return nc


def _prepare_inputs(x, W1, b1, W2, b2, batch):
    x = np.asarray(x)
    batch = np.asarray(batch).astype(np.int64)
    # core k owns segments [64k, 64(k+1)); sorted batch -> contiguous ranges
    bounds = np.searchsorted(batch, np.arange(0, NUM_GRAPHS + 1, SEGS_PER_CORE))
    counts = np.diff(bounds)
    nmax = int(np.max(counts))
    n_tiles = max(1, (nmax + TILE_N - 1) // TILE_N)
    nmax_pad = n_tiles * TILE_N

    x_bf = x.astype(ml_dtypes.bfloat16)
    w1_bf = np.asarray(W1, np.float32).astype(ml_dtypes.bfloat16)
    w2_bf = np.asarray(W2, np.float32).reshape(H, 1).astype(ml_dtypes.bfloat16)
    b1_col = np.asarray(b1, np.float32).reshape(H, 1)

    in_maps = []
    for k in range(N_CORES):
        lo, hi = int(bounds[k]), int(bounds[k + 1])
        cnt = hi - lo
        xp = np.zeros((nmax_pad, D), ml_dtypes.bfloat16)
        xp[:cnt] = x_bf[lo:hi]
        xin = np.empty((n_tiles, P, TW), ml_dtypes.bfloat16)
        # node-partitioned [t, p, ch] + ones column
        xin[:, :, 0:D] = xp.reshape(n_tiles, P, D)
        xin[:, :, D] = 1.0
        # channel-partitioned [t, p(ch in half), half, node]
        xin[:, :, DW:TW] = xp.reshape(n_tiles, TILE_N, 2, P).transpose(
            0, 3, 2, 1).reshape(n_tiles, P, D)
        # chunk-major DRAM blocks: [c, p, t, w] contiguous per chunk
        n_full = n_tiles // CHUNK_T
        t_tail = n_tiles % CHUNK_T
        m = {"w1": w1_bf, "w2": w2_bf, "b1": b1_col}
        if n_full:
            m["xin"] = np.ascontiguousarray(
                xin[:n_full * CHUNK_T].reshape(n_full, CHUNK_T, P, TW)
                .transpose(0, 2, 1, 3)).reshape(n_full * P, CHUNK_T * TW)
        if t_tail:
            m["xtail"] = np.ascontiguousarray(
                xin[n_full * CHUNK_T:].transpose(1, 0, 2)
            ).reshape(P, t_tail * TW)

        bt = np.full((nmax_pad,), -1, np.int32)
        bt[:cnt] = batch[lo:hi] - k * SEGS_PER_CORE
        bt_t = bt.reshape(n_tiles, P).T  # (128, n_tiles)
        iota_cols = np.tile(np.arange(SEGS_PER_CORE, dtype=np.int32), (P, 1))
        bt_t = np.concatenate([bt_t, iota_cols], axis=1).copy()
        m["batch_t"] = bt_t
        in_maps.append(m)
    return in_maps, n_tiles


def run(x, W1, b1, W2, b2, batch, trace=False, trace_kwargs=None):
    in_maps, n_tiles = _prepare_inputs(x, W1, b1, W2, b2, batch)
    nc = _build_program(n_tiles, float(np.asarray(b2).reshape(-1)[0]))
    nc.finalize()
    res = run_bass_kernel_spmd(nc, in_maps, list(range(N_CORES)),
                               trace=trace, **(trace_kwargs or {}))
    out = np.concatenate([np.asarray(res.results[k]["out_g"], np.float32)
                          for k in range(N_CORES)], axis=0)
    return out, res


def kernel(x, W1, b1, W2, b2, batch):
    out, _ = run(x, W1, b1, W2, b2, batch)
    return out


# revision 9
# speedup vs baseline: 1.3116x; 1.2790x over previous
"""AttentionPooling (segment softmax + weighted segment sum) on 8 trn2 cores.

Strategy: shard whole segments across cores (sorted batch -> contiguous node
ranges).  Host pre-casts x to bf16 and uploads BOTH orientations per core as
chunk-major contiguous blocks (two tensors -> two concurrent DMA streams):
  xn [c, p, t, 257]   node-partitioned (col 256 = ones, yields denominators)
  xt [c, p, 2, 4096]  channel-partitioned (for the score matmul)
so the device reads 64MB of contiguous bf16 per core and does no cast,
bounce, or transpose on chip.  Per 4096-node chunk: PE computes
h = tanh(xT @ W1 + b1) (hidden-partitioned), per-tile score columns
s = h_tile.T @ W2, ACT exponentiates, DVE builds we = onehot(batch)*e, and
PE accumulates [64,257] = we.T @ [x | 1] in PSUM across all chunks
(column 256 = softmax denominators).  wsum(c-1) is emitted between h(c) and
score(c) so PE never stalls on the tanh tail.  The tail chunk is partial
(variable tile count) so node padding is <1%.  Softmax max-subtraction is
skipped: |s| <= ||W2||_1 + |b2| ~ 28, exp stays in fp32 range.
"""

from contextlib import ExitStack

import numpy as np
import ml_dtypes

import concourse.bass as bass
import concourse.bacc as bacc
import concourse.tile as tile
from concourse import mybir
from concourse.bass_utils import run_bass_kernel_spmd

N_CORES = 8
NUM_GRAPHS = 512
SEGS_PER_CORE = NUM_GRAPHS // N_CORES  # 64
D = 256          # in channels
H = 128          # hidden
P = 128          # partitions
TILE_N = 128     # nodes per weight tile
CHUNK_T = 32     # max tiles per chunk
CHUNK_N = TILE_N * CHUNK_T  # 4096 nodes per full chunk
DW = D + 1       # node-partitioned row width: 256 channels + ones column

_BF16 = mybir.dt.bfloat16
_F32 = mybir.dt.float32
_I32 = mybir.dt.int32


def _build_program(n_tiles: int, b2_val: float):
    nc = bacc.Bacc()
    chunks = [CHUNK_T] * (n_tiles // CHUNK_T)
    if n_tiles % CHUNK_T:
        chunks.append(n_tiles % CHUNK_T)
    n_chunks = len(chunks)
    n_full = n_tiles // CHUNK_T
    t_tail = n_tiles % CHUNK_T

    xn_d = xt_d = xn_tl_d = xt_tl_d = None
    if n_full:
        xn_d = nc.declare_dram_parameter(
            "xn", [n_full * P, CHUNK_T * DW], _BF16, isOutput=False)
        xt_d = nc.declare_dram_parameter(
            "xt", [n_full * P, 2 * CHUNK_N], _BF16, isOutput=False)
    if t_tail:
        xn_tl_d = nc.declare_dram_parameter(
            "xn_tail", [P, t_tail * DW], _BF16, isOutput=False)
        xt_tl_d = nc.declare_dram_parameter(
            "xt_tail", [P, 2 * t_tail * TILE_N], _BF16, isOutput=False)
    bt_d = nc.declare_dram_parameter("batch_t", [P, n_tiles + SEGS_PER_CORE],
                                     _I32, isOutput=False)
    w1_d = nc.declare_dram_parameter("w1", [D, H], _BF16, isOutput=False)
    w2_d = nc.declare_dram_parameter("w2", [H, 1], _BF16, isOutput=False)
    b1_d = nc.declare_dram_parameter("b1", [H, 1], _F32, isOutput=False)
    out_d = nc.declare_dram_parameter("out_g", [SEGS_PER_CORE, D], _F32,
                                      isOutput=True)

    xn_ap = xn_d[:].rearrange("(c p) (t w) -> c p t w", p=P, w=DW) \
        if xn_d is not None else None
    xt_ap = xt_d[:].rearrange("(c p) (h n) -> c p h n", p=P, h=2) \
        if xt_d is not None else None
    xn_tl_ap = xn_tl_d[:].rearrange("p (t w) -> p t w", w=DW) \
        if xn_tl_d is not None else None
    xt_tl_ap = xt_tl_d[:].rearrange("p (h n) -> p h n", h=2) \
        if xt_tl_d is not None else None

    with tile.TileContext(nc) as tc, ExitStack() as ctx:
        const_pool = ctx.enter_context(tc.tile_pool(name="consts", bufs=1))
        xn_pool = ctx.enter_context(tc.tile_pool(name="xn", bufs=3))
        xt_pool = ctx.enter_context(tc.tile_pool(name="xt", bufs=3))
        h_pool = ctx.enter_context(tc.tile_pool(name="h", bufs=2))
        cmp_pool = ctx.enter_context(tc.tile_pool(name="cmp", bufs=2))
        we_pool = ctx.enter_context(tc.tile_pool(name="we", bufs=2))
        ecol_pool = ctx.enter_context(tc.tile_pool(name="ecol", bufs=2))
        fin_pool = ctx.enter_context(tc.tile_pool(name="fin", bufs=1))
        psum_h = ctx.enter_context(
            tc.tile_pool(name="psum_h", bufs=2, space=bass.MemorySpace.PSUM))
        psum_s = ctx.enter_context(
            tc.tile_pool(name="psum_s", bufs=2, space=bass.MemorySpace.PSUM))
        psum_acc = ctx.enter_context(
            tc.tile_pool(name="psum_acc", bufs=1, space=bass.MemorySpace.PSUM))

        # ---- constants / weights ----
        w1_sb = const_pool.tile([P, 2, H], _BF16, tag="w1")   # [:, 0, :]=ch 0-127
        nc.sync.dma_start(w1_sb[:, 0, :], w1_d[0:128, :])
        nc.sync.dma_start(w1_sb[:, 1, :], w1_d[128:256, :])
        w2_sb = const_pool.tile([P, 1], _BF16, tag="w2")
        nc.sync.dma_start(w2_sb[:], w2_d[:])
        b1_sb = const_pool.tile([P, 1], _F32, tag="b1")
        nc.sync.dma_start(b1_sb[:], b1_d[:])
        bt_sb = const_pool.tile([P, n_tiles + SEGS_PER_CORE], _I32, tag="bt")
        nc.sync.dma_start(bt_sb[:], bt_d[:])
        iota_sb = bt_sb[:, n_tiles:n_tiles + SEGS_PER_CORE]

        acc_ps = psum_acc.tile([SEGS_PER_CORE, DW], _F32, tag="acc")

        saved = {}

        def emit_load_h(c, t0, tc_):
            xn_t = xn_pool.tile([P, CHUNK_T, DW], _BF16, tag="xn")
            xt_t = xt_pool.tile([P, 2, CHUNK_N], _BF16, tag="xt")
            if c < n_full:
                nc.sync.dma_start(xn_t[:, 0:tc_, :], xn_ap[c])
                nc.scalar.dma_start(xt_t[:, :, 0:tc_ * TILE_N], xt_ap[c])
            else:
                nc.sync.dma_start(xn_t[:, 0:tc_, :], xn_tl_ap)
                nc.scalar.dma_start(xt_t[:, :, 0:tc_ * TILE_N], xt_tl_ap)

            # h = tanh(x @ W1 + b1), hidden-partitioned, bf16
            h_bf = h_pool.tile([P, CHUNK_N], _BF16, tag="h")
            for s0 in range(0, tc_ * TILE_N, 512):
                sn = min(512, tc_ * TILE_N - s0)
                ph = psum_h.tile([P, 512], _F32, tag="ph")
                nc.tensor.matmul(ph[:, 0:sn], w1_sb[:, 0, :],
                                 xt_t[:, 0, s0:s0 + sn],
                                 start=True, stop=False)
                nc.tensor.matmul(ph[:, 0:sn], w1_sb[:, 1, :],
                                 xt_t[:, 1, s0:s0 + sn],
                                 start=False, stop=True)
                nc.scalar.activation(h_bf[:, s0:s0 + sn], ph[:, 0:sn],
                                     mybir.ActivationFunctionType.Tanh,
                                     bias=b1_sb[:])
            saved[c] = (xn_t, h_bf)

        def emit_score(c, t0, tc_):
            xn_t, h_bf = saved[c]
            # per-tile score columns: s_col[p, t] = h_tile.T @ W2
            ps_s = psum_s.tile([P, CHUNK_T], _F32, tag="ps_s")
            for t in range(tc_):
                nc.tensor.matmul(ps_s[:, t:t + 1],
                                 h_bf[:, t * TILE_N:(t + 1) * TILE_N],
                                 w2_sb, start=True, stop=True)

            # e = exp(s + b2)  (node-partitioned, fp32)
            e_col = ecol_pool.tile([P, CHUNK_T], _F32, tag="ecol")
            nc.scalar.activation(e_col[:, 0:tc_], ps_s[:, 0:tc_],
                                 mybir.ActivationFunctionType.Exp,
                                 bias=float(b2_val))

            # we[p, t, g] = (batch_t == g) * e   (bf16)
            cmp = cmp_pool.tile([P, CHUNK_T, SEGS_PER_CORE], _BF16, tag="cmp")
            bt_c = bt_sb[:, t0:t0 + tc_]
            nc.vector.tensor_tensor(
                cmp[:, 0:tc_],
                bt_c.unsqueeze(2).broadcast_to([P, tc_, SEGS_PER_CORE]),
                iota_sb.unsqueeze(1).broadcast_to([P, tc_, SEGS_PER_CORE]),
                mybir.AluOpType.is_equal)
            we = we_pool.tile([P, CHUNK_T, SEGS_PER_CORE], _BF16, tag="we")
            nc.vector.tensor_tensor(
                we[:, 0:tc_], cmp[:, 0:tc_],
                e_col[:, 0:tc_].unsqueeze(2).broadcast_to(
                    [P, tc_, SEGS_PER_CORE]),
                mybir.AluOpType.mult)
            saved[c] = (xn_t, we)

        def emit_wsum(c, tc_, first, last):
            xn_t, we = saved.pop(c)
            for t in range(tc_):
                nc.tensor.matmul(acc_ps[:], we[:, t, :], xn_t[:, t, :],
                                 start=(first and t == 0),
                                 stop=(last and t == tc_ - 1),
                                 skip_group_check=True)

        t0s = np.concatenate([[0], np.cumsum(chunks)]).astype(int)
        for c in range(n_chunks):
            emit_load_h(c, int(t0s[c]), chunks[c])
            if c >= 1:
                emit_wsum(c - 1, chunks[c - 1], c - 1 == 0, False)
            emit_score(c, int(t0s[c]), chunks[c])
        emit_wsum(n_chunks - 1, chunks[-1], n_chunks == 1, True)

        # ---- epilogue: out = acc[:, :256] / acc[:, 256] ----
        den_sb = fin_pool.tile([SEGS_PER_CORE, 1], _F32, tag="den_sb")
        nc.vector.tensor_scalar_add(den_sb[:], acc_ps[:, D:DW], 1e-30)
        rec_sb = fin_pool.tile([SEGS_PER_CORE, 1], _F32, tag="rec_sb")
        nc.vector.reciprocal(rec_sb[:], den_sb[:])
        out_sb = fin_pool.tile([SEGS_PER_CORE, D], _F32, tag="out_sb")
        nc.vector.tensor_scalar_mul(out_sb[:], acc_ps[:, 0:D], rec_sb[:])
        nc.sync.dma_start(out_d[:], out_sb[:])

    return nc


def _prepare_inputs(x, W1, b1, W2, b2, batch):
    x = np.asarray(x)
    batch = np.asarray(batch).astype(np.int64)
    # core k owns segments [64k, 64(k+1)); sorted batch -> contiguous ranges
    bounds = np.searchsorted(batch, np.arange(0, NUM_GRAPHS + 1, SEGS_PER_CORE))
    counts = np.diff(bounds)
    nmax = int(np.max(counts))
    n_tiles = max(1, (nmax + TILE_N - 1) // TILE_N)
    nmax_pad = n_tiles * TILE_N
    n_full = n_tiles // CHUNK_T
    t_tail = n_tiles % CHUNK_T

    x_bf = x.astype(ml_dtypes.bfloat16)
    w1_bf = np.asarray(W1, np.float32).astype(ml_dtypes.bfloat16)
    w2_bf = np.asarray(W2, np.float32).reshape(H, 1).astype(ml_dtypes.bfloat16)
    b1_col = np.asarray(b1, np.float32).reshape(H, 1)

    in_maps = []
    for k in range(N_CORES):
        lo, hi = int(bounds[k]), int(bounds[k + 1])
        cnt = hi - lo
        xp = np.zeros((nmax_pad, D), ml_dtypes.bfloat16)
        xp[:cnt] = x_bf[lo:hi]
        # node-partitioned [t, p, ch] + ones column
        xn = np.empty((n_tiles, P, DW), ml_dtypes.bfloat16)
        xn[:, :, 0:D] = xp.reshape(n_tiles, P, D)
        xn[:, :, D] = 1.0
        # channel-partitioned [t, p(ch in half), half, node-in-tile]
        xt = np.ascontiguousarray(
            xp.reshape(n_tiles, TILE_N, 2, P).transpose(0, 3, 2, 1))

        m = {"w1": w1_bf, "w2": w2_bf, "b1": b1_col}
        if n_full:
            nf = n_full * CHUNK_T
            m["xn"] = np.ascontiguousarray(
                xn[:nf].reshape(n_full, CHUNK_T, P, DW).transpose(0, 2, 1, 3)
            ).reshape(n_full * P, CHUNK_T * DW)
            # per chunk: [p, half, node-in-chunk]
            m["xt"] = np.ascontiguousarray(
                xt[:nf].reshape(n_full, CHUNK_T, P, 2, TILE_N)
                .transpose(0, 2, 3, 1, 4)
            ).reshape(n_full * P, 2 * CHUNK_N)
        if t_tail:
            nf = n_full * CHUNK_T
            m["xn_tail"] = np.ascontiguousarray(
                xn[nf:].transpose(1, 0, 2)).reshape(P, t_tail * DW)
            m["xt_tail"] = np.ascontiguousarray(
                xt[nf:].transpose(1, 2, 0, 3)
            ).reshape(P, 2 * t_tail * TILE_N)

        bt = np.full((nmax_pad,), -1, np.int32)
        bt[:cnt] = batch[lo:hi] - k * SEGS_PER_CORE
        bt_t = bt.reshape(n_tiles, P).T  # (128, n_tiles)
        iota_cols = np.tile(np.arange(SEGS_PER_CORE, dtype=np.int32), (P, 1))
        bt_t = np.concatenate([bt_t, iota_cols], axis=1).copy()
        m["batch_t"] = bt_t
        in_maps.append(m)
    return in_maps, n_tiles


def run(x, W1, b1, W2, b2, batch, trace=False, trace_kwargs=None):
    in_maps, n_tiles = _prepare_inputs(x, W1, b1, W2, b2, batch)
    nc = _build_program(n_tiles, float(np.asarray(b2).reshape(-1)[0]))
    nc.finalize()
    res = run_bass_kernel_spmd(nc, in_maps, list(range(N_CORES)),
                               trace=trace, **(trace_kwargs or {}))
    out = np.concatenate([np.asarray(res.results[k]["out_g"], np.float32)
                          for k in range(N_CORES)], axis=0)
    return out, res


def kernel(x, W1, b1, W2, b2, batch):
    out, _ = run(x, W1, b1, W2, b2, batch)
    return out
